# revision 1
# baseline (speedup 1.0000x reference)
# Multi-headed attention (B=8, S=1024, D=1024, H=16) on 8 TRN2 NeuronCores.
# Strategy: pure batch data-parallel (one batch element per core, no
# collectives), all matmuls bf16 with fp32 PSUM accumulation.
# ~196us HW exec (vs 430us for the staged baseline); rel err ~5e-3.
#
# Key optimizations over the naive phased version:
#   - masked key positions are dropped on the host: key/value are gathered to
#     the unmasked positions (padded to a multiple of 128; an exp bias of
#     -30000 zeroes the pads exactly, matching the reference's -1e9 mask).
#     With the ~50% random mask this cuts scores/exp/PV and the K/V
#     projections by ~3/8. The program is compiled per padded chunk count
#     (nkc) and cached, so any mask density works.
#   - weights are pre-banded on the host so every DMA is a plain contiguous
#     [128, N] block transfer (no gpsimd gather DMAs), emitted in consumption
#     order (K-projection inputs first).
#   - the Q projection of pair p+1 is interleaved into the attention of pair
#     p, so the PE always has independent work while the scalar engine does
#     exp — otherwise the HAM clock gate re-throttles the PE to 1.2 GHz.
#   - dummy warmup matmuls cover the initial DMA window (and the final pair,
#     which has no projection work left) to keep the clock gate open.
#   - softmax denominators come free via a ones-column in the V tiles; the
#     reciprocal uses the single-pass custom-DVE reciprocal_approx_fast
#     (the iterative InstReciprocal was 3.3us per call and serialized the
#     attention pipeline). Its input must sit at partition 0 — the custom op
#     ignores the AP partition offset — hence the den row copy.
#   - the output bias (bv @ Wo + bo) is added on-device during the output
#     projection evacuation.
import math
import sys

sys.path.insert(0, "/opt/trn_rl_repo")

from contextlib import ExitStack

import ml_dtypes
import numpy as np

import concourse.bass as bass
import concourse.mybir as mybir
from concourse import bacc
from concourse import tile
from concourse.bass_utils import run_bass_kernel_spmd

dt = mybir.dt
AF = mybir.ActivationFunctionType

B, S, D, H, DK = 8, 1024, 1024, 16, 64
P = 128
NCH = D // P  # 8 chunks of 128 along the 1024-sized dims
NPAIR = H // 2  # 8 head pairs
NEGB = -30000.0  # exp underflows to exactly 0.0, matching the -1e9 masking

_NC_CACHE = {}


def build_nc(nkc: int):
    SK = nkc * P  # gathered+padded key length
    SK2 = SK // 2
    # dense-mask fallback (nkc >= 7): shallower stream buffers to fit SBUF
    lean = nkc >= 7
    ET_BUFS = 2 if lean else 4
    OB_BUFS = 2 if lean else 3
    CH_BUFS = 1 if lean else 2
    nc = bacc.Bacc()
    qT = nc.dram_tensor("qT", [D, S], dt.bfloat16, kind="ExternalInput")
    kTg = nc.dram_tensor("kTg", [D, SK], dt.bfloat16, kind="ExternalInput")
    vgb = nc.dram_tensor("vgb", [SK, D], dt.bfloat16, kind="ExternalInput")
    wqb = nc.dram_tensor("wqb", [D, D], dt.bfloat16, kind="ExternalInput")
    wkb = nc.dram_tensor("wkb", [D, D], dt.bfloat16, kind="ExternalInput")
    wv = nc.dram_tensor("wv", [D, D], dt.bfloat16, kind="ExternalInput")
    wo = nc.dram_tensor("wo", [D, D], dt.bfloat16, kind="ExternalInput")
    bq = nc.dram_tensor("bq", [P, NCH], dt.float32, kind="ExternalInput")
    bk = nc.dram_tensor("bk", [P, NCH], dt.float32, kind="ExternalInput")
    msk = nc.dram_tensor("msk", [P, nkc], dt.float32, kind="ExternalInput")
    bo = nc.dram_tensor("bo", [1, D], dt.float32, kind="ExternalInput")
    out = nc.dram_tensor("out", [S, D], dt.float32, kind="ExternalOutput")

    with tile.TileContext(nc) as tc, ExitStack() as ctx:
        big = ctx.enter_context(tc.tile_pool(name="big", bufs=NCH))
        vp = ctx.enter_context(tc.tile_pool(name="vp", bufs=nkc))
        strm = ctx.enter_context(tc.tile_pool(name="strm", bufs=4))
        one = ctx.enter_context(tc.tile_pool(name="one", bufs=1))
        psp = ctx.enter_context(tc.tile_pool(name="psp", bufs=2, space="PSUM"))

        # critical-path loads first: KT(0) needs wkb band 0 + all of kTg
        wkb_sb = [None] * NPAIR

        def load_wkb(p):
            t = big.tile([P, D], dt.bfloat16, tag="wkb")
            nc.sync.dma_start(t[:], wkb[p * P : (p + 1) * P, :])
            wkb_sb[p] = t

        load_wkb(0)
        xk = []
        for di in range(NCH):
            t = big.tile([P, SK], dt.bfloat16, tag="xk")
            nc.sync.dma_start(t[:], kTg[di * P : (di + 1) * P, :])
            xk.append(t)

        # PE warmup: small dummy matmuls on a zeroed scratch tile keep the
        # HAM activity window busy while the first DMAs land, so real work
        # starts at the full 2.4 GHz clock.
        scr = one.tile([P, 512], dt.bfloat16, tag="scr")
        nc.gpsimd.memset(scr[:], 0.0)
        wps = psp.tile([P, 512], dt.float32, tag="proj")
        for _ in range(80):
            nc.tensor.matmul(
                wps[:, 0:P], scr[:, 0:P], scr[:, 512 - P : 512], start=True, stop=True
            )

        # small constants
        msk_sb = one.tile([P, nkc], dt.float32, tag="msk")
        nc.sync.dma_start(msk_sb[:], msk[:])
        bq_sb = one.tile([P, NCH], dt.float32, tag="bq")
        nc.sync.dma_start(bq_sb[:], bq[:])
        bk_sb = one.tile([P, NCH], dt.float32, tag="bk")
        nc.sync.dma_start(bk_sb[:], bk[:])
        bo_row = one.tile([1, D], dt.float32, tag="bo_row")
        nc.sync.dma_start(bo_row[:], bo[:])

        # warm the ACT exp table while DMAs stream
        warm = one.tile([1, nkc], dt.float32, tag="warm")
        nc.scalar.activation(warm[:], msk_sb[0:1, :], AF.Exp, bias=0.0, scale=1.0)

        # output-bias row broadcast to all partitions (used in phase 3)
        bo_sb = one.tile([P, D], dt.float32, tag="bo_sb")
        nc.gpsimd.partition_broadcast(bo_sb[:], bo_row[:])

        # remaining weight bands
        for p in range(1, NPAIR):
            load_wkb(p)
        vgb_sb = []
        for kc in range(nkc):
            t = vp.tile([P, D], dt.bfloat16, tag="vgb")
            nc.sync.dma_start(t[:], vgb[kc * P : (kc + 1) * P, :])
            vgb_sb.append(t)
        wv_sb = []
        for di in range(NCH):
            t = big.tile([P, D], dt.bfloat16, tag="wv")
            nc.sync.dma_start(t[:], wv[di * P : (di + 1) * P, :])
            wv_sb.append(t)
        xq = []
        for di in range(NCH):
            t = big.tile([P, S], dt.bfloat16, tag="xq")
            nc.sync.dma_start(t[:], qT[di * P : (di + 1) * P, :])
            xq.append(t)
        wqb_sb = []
        for p in range(NPAIR):
            t = big.tile([P, D], dt.bfloat16, tag="wqb")
            nc.sync.dma_start(t[:], wqb[p * P : (p + 1) * P, :])
            wqb_sb.append(t)
        wo_sb = []
        for pc in range(NCH):
            t = big.tile([P, D], dt.bfloat16, tag="wo")
            nc.sync.dma_start(t[:], wo[pc * P : (pc + 1) * P, :])
            wo_sb.append(t)

        # ---- K projection (all pairs), [d, s] layout -------------------
        kt_t = []
        for p in range(NPAIR):
            t = big.tile([P, SK], dt.bfloat16, tag="kt")
            kt_t.append(t)
            for half in range(2):
                hs = slice(half * SK2, (half + 1) * SK2)
                ps = psp.tile([P, 512], dt.float32, tag="proj", name=f"kt_ps{p}_{half}")
                for di in range(NCH):
                    nc.tensor.matmul(
                        ps[:, 0:SK2],
                        wkb_sb[p][:, di * P : (di + 1) * P],
                        xk[di][:, hs],
                        start=(di == 0),
                        stop=(di == NCH - 1),
                    )
                nc.vector.tensor_scalar_add(t[:, hs], ps[:, 0:SK2], bk_sb[:, p : p + 1])

        # ---- V projection -> natural [s, 16*65] with ones columns ------
        vv_t = []
        for kc in range(nkc):
            t = vp.tile([P, H * (DK + 1)], dt.bfloat16, tag="vv")
            vv_t.append(t)
            nc.gpsimd.memset(t[:], 1.0)
            for half in range(2):
                hs = slice(half * 512, (half + 1) * 512)
                ps = psp.tile([P, 512], dt.float32, tag="proj", name=f"v_ps{kc}_{half}")
                for di in range(NCH):
                    nc.tensor.matmul(
                        ps[:],
                        vgb_sb[kc][:, di * P : (di + 1) * P],
                        wv_sb[di][:, hs],
                        start=(di == 0),
                        stop=(di == NCH - 1),
                    )
                dst = t[:, half * 520 : half * 520 + 520].rearrange(
                    "p (h e) -> p h e", e=DK + 1
                )[:, :, 0:DK]
                srcv = ps[:].rearrange("p (h e) -> p h e", e=DK)
                nc.vector.tensor_copy(dst, srcv)

        # ---- Q projection helper (quarter granularity for interleave) ---
        qt_t = [None] * NPAIR

        def emit_qt_quarter(p, quarter):
            if qt_t[p] is None:
                qt_t[p] = big.tile([P, S], dt.bfloat16, tag="qt", name=f"qt{p}")
            t = qt_t[p]
            qs = slice(quarter * 256, (quarter + 1) * 256)
            ps = psp.tile([P, 512], dt.float32, tag="proj", name=f"qt_ps{p}_{quarter}")
            for di in range(NCH):
                nc.tensor.matmul(
                    ps[:, 0:256],
                    wqb_sb[p][:, di * P : (di + 1) * P],
                    xq[di][:, qs],
                    start=(di == 0),
                    stop=(di == NCH - 1),
                )
            nc.vector.tensor_scalar_add(t[:, qs], ps[:, 0:256], bq_sb[:, p : p + 1])

        for quarter in range(4):
            emit_qt_quarter(0, quarter)

        # ---- attention per head pair, Q proj of p+1 interleaved --------
        ct_t = []
        out_pre = {}
        for p in range(NPAIR):
            ct = big.tile([P, S], dt.bfloat16, tag="ct")
            ct_t.append(ct)
            for hq in range(2):
                qs = slice(hq * 512, (hq + 1) * 512)
                pv0 = psp.tile([DK + 1, 512], dt.float32, tag="pv")
                pv1 = psp.tile([DK + 1, 512], dt.float32, tag="pv")
                for kc in range(nkc):
                    if kc == min(2, nkc - 1) and p + 1 < NPAIR:
                        # PE filler while ACT grinds exp: two quarters of the
                        # next pair's Q projection per attention half
                        emit_qt_quarter(p + 1, 2 * hq)
                        emit_qt_quarter(p + 1, 2 * hq + 1)
                    if p == NPAIR - 1 and kc >= 1:
                        # no projection work left for the last pair: keep the
                        # HAM activity window open with dummy matmuls so the
                        # real matmuls stay at full clock
                        dps = psp.tile([P, 512], dt.float32, tag="proj", name=f"d{hq}_{kc}")
                        for _ in range(4):
                            nc.tensor.matmul(
                                dps[:, 0:P],
                                scr[:, 0:P],
                                scr[:, 512 - P : 512],
                                start=True,
                                stop=True,
                            )
                    st = psp.tile([P, 1024], dt.float32, tag="st")
                    ks = slice(kc * P, (kc + 1) * P)
                    nc.tensor.matmul(
                        st[:, 0:512],
                        kt_t[p][0:DK, ks],
                        qt_t[p][0:DK, qs],
                        start=True,
                        stop=True,
                        tile_position=(0, 0),
                    )
                    nc.tensor.matmul(
                        st[:, 512:1024],
                        kt_t[p][DK:P, ks],
                        qt_t[p][DK:P, qs],
                        start=True,
                        stop=True,
                        tile_position=(DK, 0),
                    )
                    et = strm.tile([P, 1024], dt.bfloat16, tag="et", bufs=ET_BUFS)
                    nc.scalar.activation(
                        et[:], st[:], AF.Exp, bias=msk_sb[:, kc : kc + 1], scale=1.0
                    )
                    nc.tensor.matmul(
                        pv0[:],
                        vv_t[kc][:, (2 * p) * 65 : (2 * p) * 65 + 65],
                        et[:, 0:512],
                        start=(kc == 0),
                        stop=(kc == nkc - 1),
                    )
                    nc.tensor.matmul(
                        pv1[:],
                        vv_t[kc][:, (2 * p + 1) * 65 : (2 * p + 1) * 65 + 65],
                        et[:, 512:1024],
                        start=(kc == 0),
                        stop=(kc == nkc - 1),
                    )
                if p == NPAIR - 1 and hq == 1:
                    # prefill the first output-projection column block with
                    # head pairs 0..6 while pair 7's softmax chain drains;
                    # the pc=7 finisher runs in the output phase
                    for half in range(2):
                        pso = psp.tile(
                            [P, 512], dt.float32, tag="proj", name=f"o_pre{half}"
                        )
                        out_pre[half] = pso
                        for pc in range(NCH - 1):
                            nc.tensor.matmul(
                                pso[:],
                                ct_t[pc][:, 0:P],
                                wo_sb[pc][:, half * 512 : (half + 1) * 512],
                                start=(pc == 0),
                                stop=False,
                            )
                for hloc, pv in ((0, pv0), (1, pv1)):
                    den = strm.tile([1, 512], dt.float32, tag="den", bufs=CH_BUFS)
                    nc.vector.tensor_copy(den[:], pv[DK : DK + 1, :])
                    rcp = strm.tile([1, 512], dt.float32, tag="rcp", bufs=CH_BUFS)
                    nc.vector.reciprocal_approx_fast(rcp[:], den[:])
                    rb = strm.tile([DK, 512], dt.float32, tag="rb", bufs=CH_BUFS)
                    nc.gpsimd.partition_broadcast(rb[:], rcp[:])
                    nc.vector.tensor_mul(
                        ct[hloc * DK : (hloc + 1) * DK, qs], pv[0:DK, :], rb[:]
                    )

        # ---- output projection + bias ----------------------------------
        for qc in range(NCH):
            for half in range(2):
                hs = slice(half * 512, (half + 1) * 512)
                if qc == 0 and half in out_pre:
                    ps = out_pre[half]
                    nc.tensor.matmul(
                        ps[:],
                        ct_t[NCH - 1][:, 0:P],
                        wo_sb[NCH - 1][:, hs],
                        start=False,
                        stop=True,
                    )
                else:
                    ps = psp.tile(
                        [P, 512], dt.float32, tag="proj", name=f"o_ps{qc}_{half}"
                    )
                    for pc in range(NCH):
                        nc.tensor.matmul(
                            ps[:],
                            ct_t[pc][:, qc * P : (qc + 1) * P],
                            wo_sb[pc][:, hs],
                            start=(pc == 0),
                            stop=(pc == NCH - 1),
                        )
                ob = strm.tile([P, 512], dt.float32, tag="ob", bufs=OB_BUFS)
                nc.vector.tensor_add(ob[:], ps[:], bo_sb[:, hs])
                nc.sync.dma_start(out[qc * P : (qc + 1) * P, hs], ob[:])

    nc.finalize()
    return nc


def _band(w: np.ndarray, ncol: int) -> np.ndarray:
    # w: [1024, ncol*128]. Output row-block p holds column-band p rearranged
    # as [128 rows (r), 8 chunks (di) x 128]: out[p*128+r, di*128+c] =
    # w[di*128+r, p*128+c]  -- the stationary layout for lhsT slices.
    return np.ascontiguousarray(
        w.reshape(NCH, P, ncol, P).transpose(2, 1, 0, 3).reshape(ncol * P, D)
    )


def _make_in_maps(query, key, value, mask, Wq, bq, Wk, bk, Wv, bv, Wo, bo):
    query = np.asarray(query, dtype=np.float32)
    key = np.asarray(key, dtype=np.float32)
    value = np.asarray(value, dtype=np.float32)
    mask = np.asarray(mask)
    Wq = np.asarray(Wq, dtype=np.float32)
    Wk = np.asarray(Wk, dtype=np.float32)
    Wv = np.asarray(Wv, dtype=np.float32)
    Wo = np.asarray(Wo, dtype=np.float32)
    sc = np.float32(1.0 / math.sqrt(DK))
    bo_eff = (np.asarray(bv, np.float32) @ Wo + np.asarray(bo, np.float32)).reshape(
        1, D
    )

    idxs, nv = [], []
    for i in range(B):
        ix = np.nonzero(np.asarray(mask[i, 0]) != 0)[0]
        idxs.append(ix)
        nv.append(len(ix))
    nkc = min(NCH, max(1, -(-max(nv) // P)))
    SK = nkc * P

    bf16 = ml_dtypes.bfloat16
    wqb = _band(Wq * sc, NCH).astype(bf16)
    wkb = _band(Wk, NCH).astype(bf16)
    wv_b = np.ascontiguousarray(Wv).astype(bf16)
    wo_b = np.ascontiguousarray(Wo).astype(bf16)
    bq2 = np.ascontiguousarray((np.asarray(bq, np.float32) * sc).reshape(NCH, P).T)
    bk2 = np.ascontiguousarray(np.asarray(bk, np.float32).reshape(NCH, P).T)

    in_maps = []
    for i in range(B):
        ix = idxs[i]
        pad = SK - len(ix)
        ixp = np.concatenate([ix, np.zeros(pad, dtype=ix.dtype)])
        mb = np.full(SK, 0.0, dtype=np.float32)
        if pad:
            mb[len(ix) :] = NEGB
        kTg = np.ascontiguousarray(key[i][ixp].astype(bf16).T)
        vT = value[i][ixp].astype(bf16).T  # [D, SK]
        vgb = np.ascontiguousarray(_band(vT, nkc))
        in_maps.append(
            {
                "qT": np.ascontiguousarray(query[i].astype(bf16).T),
                "kTg": kTg,
                "vgb": vgb,
                "wqb": wqb,
                "wkb": wkb,
                "wv": wv_b,
                "wo": wo_b,
                "bq": bq2,
                "bk": bk2,
                "msk": np.ascontiguousarray(mb.reshape(nkc, P).T),
                "bo": bo_eff,
            }
        )
    return nkc, in_maps


def kernel(query, key, value, mask, Wq, bq, Wk, bk, Wv, bv, Wo, bo):
    nkc, in_maps = _make_in_maps(
        query, key, value, mask, Wq, bq, Wk, bk, Wv, bv, Wo, bo
    )
    if nkc not in _NC_CACHE:
        _NC_CACHE[nkc] = build_nc(nkc)
    nc = _NC_CACHE[nkc]
    res = run_bass_kernel_spmd(nc, in_maps, list(range(B)))
    return np.stack([res.results[i]["out"] for i in range(B)], axis=0).astype(
        np.float32
    )

